# revision 2
# baseline (speedup 1.0000x reference)
"""Trainium2 Bass kernel for the spike-train CV (coefficient of variation) loss.

Problem: for each (batch, neuron) sequence of T=2000 time steps, spikes are
positions where x > 0.  The loss is MSE between per-sequence CV of the
inter-spike intervals (ISIs, unbiased std / mean, penalty 10.0 when fewer
than 3 spikes) and a per-neuron target.

Per (neuron, batch) sequence the device computes four exact integer stats,
all derivable in one streaming pipeline over time (laid along the SBUF free
axis):
  s/b  = sign(x) (ACT) or [x>0] (DVE)        + accum -> count encoding
  v    = s * (t+1)                           (DVE or POOL tensor_tensor)
  g    = running max of v, floored at 0      (DVE tensor_tensor_scan)
       = 1 + time of last spike <= t, 0 if none
  P    = sum_t g                             (accum pass, DVE or ACT)
  G    = sum_t [g>0] = T - first_spike       (accum pass, DVE or ACT)
  last = g(T-1) - 1                          (tiny column copy)
The host then finishes in float64: first = T-G, s1 = last-first (telescoping
ISI sum), and the run-length identity collapses the ISI square sum to
  s2 = 2T*last - last^2 - first^2 - 2*first - 2*(P - T),
then torch-style unbiased CV with the penalty-10 select, MSE against the
target, and the 8-core mean.  All stats are integers below 2^24, so the
device arithmetic (f16 tensors, f32 accumulators) is exact.

Engine budget per core (16 groups of 128 neurons): DVE owns the 16 scans
(~1.8us each, the only sequential op) plus a slice of the accumulation
passes; ACT takes most X/G/P accumulation passes (~1.85us each); POOL runs
the v multiplies (~2.1us each).  Loads go f32 -> f32 through the hardware
DGE on the sync queue (cast DMAs would need POOL's software DGE, which
would steal POOL compute time).

Sharding: batch dim (B=8) across the 8 cores, embarrassingly parallel; host
transposes each core's slab to (N, T) and reduces the per-core stats.
"""

import numpy as np

import concourse.bass as bass
import concourse.tile as tile
from concourse import mybir
from concourse.bass_utils import run_bass_kernel_spmd

B, T, N = 8, 2000, 2048
P = 128                 # SBUF partitions
NB = N // P             # 16 neuron groups per core
F32 = mybir.dt.float32
F16 = mybir.dt.float16
BF16 = mybir.dt.bfloat16
A = mybir.AluOpType
AF = mybir.ActivationFunctionType
AX = mybir.AxisListType

_CACHE = {}


def _build(n_x_act=16, n_g_act=8, n_p_act=4, n_v_pool=16,
           d1_small=True, xbufs=6, sbufs=4, vbufs=4, gbufs=4, dbufs=4,
           dl=3, xl=2, repeats=1, dma_only=False):
    """Emit the stats pipeline.

    n_x_act:  groups whose X-pass (sign+count) runs on ACT (prefix).
    n_g_act:  groups whose G-pass runs on ACT.
    n_p_act:  groups whose P-pass runs on ACT.
    n_v_pool: groups whose v-multiply runs on POOL (gpsimd).
    dl/xl:    emission lookahead, DMA ahead of X ahead of scan.
    """
    nc = bass.Bass("TRN2", target_bir_lowering=False, debug=False,
                   num_devices=B)

    xT = nc.dram_tensor("xT", [N, T], F32, kind="ExternalInput").ap()
    iota = nc.dram_tensor("iota", [P, T], F16, kind="ExternalInput").ap()
    sst_o = nc.dram_tensor("sst", [P, NB], F32, kind="ExternalOutput").ap()
    gst_o = nc.dram_tensor("gst", [P, NB], F32, kind="ExternalOutput").ap()
    pst_o = nc.dram_tensor("pst", [P, NB], F32, kind="ExternalOutput").ap()
    lst_o = nc.dram_tensor("lst", [P, NB], F32, kind="ExternalOutput").ap()

    with tile.TileContext(nc) as tc:
        with (
            tc.tile_pool(name="const", bufs=1) as const_pool,
            tc.tile_pool(name="stats", bufs=1) as stats_pool,
            tc.tile_pool(name="xload", bufs=xbufs) as xload,
            tc.tile_pool(name="sb", bufs=sbufs) as sbp,
            tc.tile_pool(name="vp", bufs=vbufs) as vp,
            tc.tile_pool(name="gp", bufs=gbufs) as gp,
            tc.tile_pool(name="dead", bufs=dbufs) as dead,
        ):
            iota_t = const_pool.tile([P, T], F16, tag="iota")
            nc.sync.dma_start(iota_t[:], iota[:])
            # Pre-touch the constant tile on DVE and POOL so downstream
            # tensor_tensor ops don't need a second (DMA) sync-wait slot.
            touch = const_pool.tile([P, 1], F16, tag="touch")
            nc.vector.tensor_copy(touch[:], iota_t[:, 0:1])

            Sst = stats_pool.tile([P, NB], F32, tag="Sst")
            Gst = stats_pool.tile([P, NB], F32, tag="Gst")
            Pst = stats_pool.tile([P, NB], F32, tag="Pst")
            lastp = stats_pool.tile([P, NB], F32, tag="lastp")

            for rep in range(repeats):
                xts = {}
                ss = {}
                gs = {}

                def emit_dma(nb):
                    xt = xload.tile([P, T], F32, tag="xt")
                    nc.sync.dma_start(xt[:], xT[nb * P:(nb + 1) * P, :])
                    xts[nb] = xt

                def emit_x(nb):
                    xt = xts.pop(nb)
                    s = sbp.tile([P, T], F16, tag="s")
                    if nb < n_x_act:
                        # s = sign(x) in {-1,+1}; accum = 2*count - T
                        nc.scalar.activation(
                            s[:], xt[:], AF.Sign,
                            accum_out=Sst[:, nb:nb + 1],
                        )
                    else:
                        # s = [x>0] in {0,1}; accum = count
                        nc.vector.tensor_scalar(
                            s[:], xt[:], 0.0, None, op0=A.is_gt, op1=A.add,
                            accum_out=Sst[:, nb:nb + 1],
                        )
                    ss[nb] = s

                def emit_rest(nb):
                    s = ss.pop(nb)
                    v = vp.tile([P, T], F16, tag="v")
                    eng = nc.gpsimd if nb < n_v_pool else nc.vector
                    eng.tensor_tensor(v[:], s[:], iota_t[:], op=A.mult)

                    g = gp.tile([P, T], F16, tag="g")
                    d1 = (touch[:, 0:1].broadcast_to([P, T])
                          if d1_small else v[:])
                    nc.vector.tensor_tensor_scan(
                        g[:], v[:], d1, 0.0, op0=A.max, op1=A.bypass
                    )
                    nc.vector.tensor_copy(
                        lastp[:, nb:nb + 1], g[:, T - 1:T]
                    )
                    dp = dead.tile([P, T], F16, tag="dp")
                    if nb < n_p_act:
                        nc.scalar.activation(
                            dp[:], g[:], AF.Copy,
                            accum_out=Pst[:, nb:nb + 1],
                        )
                    else:
                        nc.vector.tensor_scalar(
                            dp[:], g[:], 0.0, None, op0=A.add, op1=A.add,
                            accum_out=Pst[:, nb:nb + 1],
                        )
                    dg = dead.tile([P, T], F16, tag="dg")
                    if nb < n_g_act:
                        nc.scalar.activation(
                            dg[:], g[:], AF.Sign,
                            accum_out=Gst[:, nb:nb + 1],
                        )
                    else:
                        nc.vector.tensor_scalar(
                            dg[:], g[:], 1.0, None, op0=A.min, op1=A.add,
                            accum_out=Gst[:, nb:nb + 1],
                        )

                if dma_only:
                    for nb in range(NB):
                        emit_dma(nb)
                    for nb in range(NB):
                        xt = xts.pop(nb)
                        nc.vector.tensor_copy(
                            lastp[:, nb:nb + 1], xt[:, 0:1]
                        )
                    nc.vector.memset(Sst[:], 1000.0)
                    nc.vector.memset(Gst[:], 1000.0)
                    nc.vector.memset(Pst[:], 1000.0)
                    continue

                for i in range(NB + dl + xl):
                    if i < NB:
                        emit_dma(i)
                    j = i - dl
                    if 0 <= j < NB:
                        emit_x(j)
                    k = i - dl - xl
                    if 0 <= k < NB:
                        emit_rest(k)

            nc.sync.dma_start(sst_o[:], Sst[:])
            nc.sync.dma_start(gst_o[:], Gst[:])
            nc.sync.dma_start(pst_o[:], Pst[:])
            nc.sync.dma_start(lst_o[:], lastp[:])

    return nc


def _legalize_waits(nc):
    """Hoist excess sync-waits onto standalone EventSemaphore instructions.

    Hardware instruction encodings hold a single sync-wait (EventSemaphore
    holds two); the deployed tile scheduler sometimes attaches more, which
    walrus codegen rejects ("Too many sync wait commands").  Splitting the
    extra waits into preceding same-engine EventSemaphore ops is exactly
    equivalent: the engine stalls on the standalone waits first.
    """
    f = nc.m.functions[0]
    for blk in f.blocks:
        newlist = []
        for inst in blk.instructions:
            si = inst.sync_info
            tname = type(inst).__name__
            waits = list(si.on_wait) if si is not None else []
            cap = 2 if tname == "InstEventSemaphore" else 1
            if len(waits) <= cap:
                newlist.append(inst)
                continue
            for j, w in enumerate(waits[:-1]):
                es = mybir.InstEventSemaphore(name=f"{inst.name}-hw{j}")
                es.engine = inst.engine
                es.sync_info = mybir.SyncInfo(on_wait=[w], on_update=[])
                newlist.append(es)
            inst.sync_info = mybir.SyncInfo(
                on_wait=[waits[-1]], on_update=list(si.on_update)
            )
            newlist.append(inst)
        blk.instructions = newlist


def _get_nc(**flags):
    key = tuple(sorted(flags.items()))
    if key not in _CACHE:
        nc = _build(**flags)
        _legalize_waits(nc)
        _CACHE[key] = nc
    return _CACHE[key]


FLAGS = dict(n_x_act=16, n_g_act=8, n_p_act=4, n_v_pool=16)


def _host_finish(sst, gst, pst, lst, tgt_pn, n_x_act):
    """Decode per-(partition, group) stats and compute the CV loss terms.

    sst/gst/pst/lst: [P, NB] f32 per-core stats.  Returns sum of squared
    errors over this core's (P*NB) sequences, in float64.
    """
    sst = sst.astype(np.float64)
    cnt = np.empty_like(sst)
    cnt[:, :n_x_act] = (sst[:, :n_x_act] + T) * 0.5
    cnt[:, n_x_act:] = sst[:, n_x_act:]
    G = gst.astype(np.float64)
    Pv = pst.astype(np.float64)
    last = lst.astype(np.float64) - 1.0
    first = T - G

    k = cnt - 1.0
    s1 = last - first
    s2 = (2.0 * T * last - last * last - first * first - 2.0 * first
          - 2.0 * (Pv - T))
    mean = s1 / np.maximum(k, 1.0)
    var = (s2 - k * mean * mean) / np.maximum(k - 1.0, 1.0)
    std = np.sqrt(np.maximum(var, 0.0))
    cv = std / np.where(mean > 0.0, mean, 1.0)
    cvs = np.where((cnt >= 3.0) & (mean > 0.0), cv, 10.0)
    d = cvs - tgt_pn
    return float(np.sum(d * d))


def kernel(output_spikes, target_cv):
    x = np.asarray(output_spikes, dtype=np.float32)
    tgt = np.asarray(target_cv, dtype=np.float32)
    assert x.shape == (B, T, N), x.shape

    iota_np = np.broadcast_to(
        (np.arange(T, dtype=np.float32) + 1.0).astype(np.float16), (P, T)
    ).copy()
    tgt_pn = tgt.reshape(NB, P).T.astype(np.float64)  # [P, NB]

    in_maps = []
    for b in range(B):
        in_maps.append({
            "xT": np.ascontiguousarray(x[b].T),  # (N, T)
            "iota": iota_np,
        })

    nc = _get_nc(**FLAGS)
    res = run_bass_kernel_spmd(nc, in_maps, list(range(B)))

    total = 0.0
    for b in range(B):
        r = res.results[b]
        total += _host_finish(
            r["sst"], r["gst"], r["pst"], r["lst"], tgt_pn,
            FLAGS["n_x_act"],
        )
    loss = total / float(B * N)
    return np.float32(loss)


# revision 26
# speedup vs baseline: 1.3703x; 1.3703x over previous
"""Trainium2 Bass kernel for the spike-train CV (coefficient of variation) loss.

Problem: for each (batch, neuron) sequence of T=2000 time steps, spikes are
positions where x > 0.  The loss is MSE between per-sequence CV of the
inter-spike intervals (ISIs, unbiased std / mean, penalty 10.0 when fewer
than 3 spikes) and a per-neuron target.

Algorithm (per sequence, all exact integer arithmetic except one fp32 sum):
  s(t)   = sign(x)                                (ACT engine, fp16 out)
  v(t)   = s(t) * (t+1)                           (DVE tensor_tensor, fp16 2x)
  g(t)   = max(0, running max of v)               (DVE tensor_tensor_scan)
         = 1 + (time of last spike <= t), 0 if none
  count  = (sum_t s + T) / 2
  first  = T - sum_t [g>0],   last = g(T-1) - 1
  P      = sum_t g  =>  sum_t prev_incl(t) = P - T
From these, the ISI sum s1 = last-first (telescoping) and the ISI square sum
via the run-length identity:
  R      = sum_{t=first..last} (t - prev_incl(t))
  sum g^2 over internal zero-runs = 2R - Z,  Z = s1+1-count
  s2     = (2R - Z) + 2*s1 - count + 1
then cv = std/mean with torch-style unbiased variance, penalty when count<3.

Sharding: batch dim (B=8) across the 8 cores, embarrassingly parallel; host
transposes each core's slab to (N, T) so time lies along the SBUF free axis
(the scan direction) and sums the 8 per-core partial squared-error sums.
"""

import numpy as np

import concourse.bass as bass
import concourse.tile as tile
from concourse import mybir
from concourse.bass_utils import run_bass_kernel_spmd

B, T, N = 8, 2000, 2048
P = 128                 # SBUF partitions
NB = N // P             # 16 neuron groups per core
F32 = mybir.dt.float32
F16 = mybir.dt.float16
BF16 = mybir.dt.bfloat16
A = mybir.AluOpType
AF = mybir.ActivationFunctionType
AX = mybir.AxisListType

_CACHE = {}


def _build(g_on_act=False, p_via_scan_accum=False, v_engine="dve_stt",
           g_split=0, final_split=False, cast_dma=True, dma_only=False,
           p_split=0, scan_bypass=False, scan_d1_small=False, half_bufs=4, look=2,
           stage=4, repeats=1, host_final=False, v_pool_split=0,
           phase_mode=False, x_dve_split=0, g_cols_dve=0):
    nc = bass.Bass("TRN2", target_bir_lowering=False, debug=False, num_devices=B)

    xT = nc.dram_tensor("xT", [N, T], F32, kind="ExternalInput").ap()
    iota = nc.dram_tensor("iota", [P, T], F16, kind="ExternalInput").ap()
    if host_final:
        sst_o = nc.dram_tensor("sst", [P, NB], F32, kind="ExternalOutput").ap()
        gst_o = nc.dram_tensor("gst", [P, NB], F32, kind="ExternalOutput").ap()
        pst_o = nc.dram_tensor("pst", [P, NB], F32, kind="ExternalOutput").ap()
        lst_o = nc.dram_tensor("lst", [P, NB], F32, kind="ExternalOutput").ap()
    else:
        tgt = nc.dram_tensor("tgt", [P, NB], F32, kind="ExternalInput").ap()
        out = nc.dram_tensor("out", [P, 1], F32, kind="ExternalOutput").ap()

    with tile.TileContext(nc) as tc:
        with (
            tc.tile_pool(name="const", bufs=1) as const_pool,
            tc.tile_pool(name="stats", bufs=1) as stats_pool,
            tc.tile_pool(name="xload", bufs=NB) as xload,
            tc.tile_pool(name="half", bufs=half_bufs) as half,
            tc.tile_pool(name="fin", bufs=1) as fin,
        ):
            iota_t = const_pool.tile([P, T], F16, tag="iota")
            nc.gpsimd.dma_start(iota_t[:], iota[:])
            # Pre-touch the constant tile on DVE so downstream tensor_tensor
            # ops don't need a second (DMA) sync-wait slot — the TT ISA
            # struct only has one.
            touch = const_pool.tile([P, 1], F16, tag="touch")
            nc.vector.tensor_copy(touch[:], iota_t[:, 0:1])
            # tgt is only needed by the final math; load it late so its DMA
            # doesn't stall the first TT of the main loop.  Its consumer
            # (the diff TT) carries the DMA wait itself.
            if not host_final:
                tgt_t = const_pool.tile([P, NB], F32, tag="tgt")

            sum_s = stats_pool.tile([P, NB], F32, tag="sum_s")
            Pst = stats_pool.tile([P, NB], F32, tag="Pst")
            Gst = stats_pool.tile([P, NB], F32, tag="Gst")
            lastp = stats_pool.tile([P, NB], F32, tag="lastp")
            if g_cols_dve:
                Gst2 = stats_pool.tile([P, NB], F32, name="Gst2", tag="Gst2")
            else:
                Gst2 = None

            # ---- final per-neuron algebra on [P, NB] f32 tiles ----
            # Everything reduces algebraically to
            #   s2 = 2T*last - last^2 - first^2 - 2*first - 2*(P - T)
            # (the run-length identity chain collapses), then the torch-style
            # unbiased CV with penalty-10 select.
            tiles = {}

            def ft(tag):
                if tag not in tiles:
                    tiles[tag] = fin.tile([P, NB], F32, name=tag, tag=tag)
                return tiles[tag]

            def emit_final(lo, hi):
                sl = slice(lo, hi)

                def ts(out_t, in_t, s1_, s2_, op0, op1=None):
                    if op1 is None:
                        nc.vector.tensor_scalar(
                            out_t[:, sl], in_t[:, sl], s1_, None, op0=op0
                        )
                    else:
                        nc.vector.tensor_scalar(
                            out_t[:, sl], in_t[:, sl], s1_, s2_, op0=op0, op1=op1
                        )
                    return out_t

                def tt(out_t, a, b, op):
                    nc.vector.tensor_tensor(
                        out_t[:, sl], a[:, sl], b[:, sl], op=op
                    )
                    return out_t

                def stt(out_t, a, scal, b, op0, op1):
                    nc.vector.scalar_tensor_tensor(
                        out_t[:, sl], a[:, sl], scal, b[:, sl], op0=op0, op1=op1
                    )
                    return out_t

                cnt = ts(ft("cnt"), sum_s, float(T), 0.5, A.add, A.mult)
                if g_cols_dve:
                    gsum = tt(ft("gsum"), Gst, Gst2, A.add)
                    first = ts(ft("first"), gsum, -1.0, float(T),
                               A.mult, A.add)
                else:
                    first = ts(ft("first"), Gst, -1.0, float(T),
                               A.mult, A.add)
                last = ts(ft("last"), lastp, -1.0, None, A.add)
                s1 = tt(ft("s1"), last, first, A.subtract)
                k = ts(ft("k"), cnt, -1.0, None, A.add)
                h = ts(ft("h"), Pst, -2.0, 2.0 * T, A.mult, A.add)  # -2*P_full
                e1 = ts(ft("e1"), last, 2.0 * T, None, A.mult)
                bb = tt(ft("bb"), last, last, A.mult)
                aa = tt(ft("aa"), first, first, A.mult)
                g1 = tt(ft("g1"), e1, bb, A.subtract)
                g2 = tt(ft("g2"), g1, aa, A.subtract)
                g3 = ts(ft("g3"), first, -2.0, None, A.mult)
                g4 = tt(ft("g4"), g2, g3, A.add)
                s2t = tt(ft("s2t"), g4, h, A.add)

                maxk = ts(ft("maxk"), k, 1.0, None, A.max)
                invmaxk = ft("invmaxk")
                nc.vector.reciprocal(invmaxk[:, sl], maxk[:, sl])
                mean = tt(ft("mean"), s1, invmaxk, A.mult)
                km1 = ts(ft("km1"), k, -1.0, 1.0, A.add, A.max)
                invkm1 = ft("invkm1")
                nc.vector.reciprocal(invkm1[:, sl], km1[:, sl])

                # k*mean^2 == s1*mean for k>=1 (maxk==k); for the masked
                # k<=0 lanes both stay finite, which is all that matters.
                km2 = tt(ft("km2"), s1, mean, A.mult)
                d = tt(ft("d"), s2t, km2, A.subtract)
                var = tt(ft("var"), d, invkm1, A.mult)
                varc = ts(ft("varc"), var, 0.0, None, A.max)
                std = ft("std")
                nc.scalar.activation(std[:, sl], varc[:, sl], AF.Sqrt)

                dm = ts(ft("dm"), mean, -1.0, None, A.add)
                t4 = stt(ft("t4"), mean, 0.0, dm, A.is_gt, A.mult)
                denom = ts(ft("denom"), t4, 1.0, None, A.add)
                invden = ft("invden")
                nc.vector.reciprocal(invden[:, sl], denom[:, sl])
                cv = tt(ft("cv"), std, invden, A.mult)

                cm = ts(ft("cm"), cv, -10.0, None, A.add)
                t5 = stt(ft("t5"), cnt, 3.0, cm, A.is_ge, A.mult)
                cvs = ts(ft("cvs"), t5, 10.0, None, A.add)

                diff = tt(ft("diff"), cvs, tgt_t, A.subtract)
                tt(ft("sq"), diff, diff, A.mult)

            emitted_halves = set()

            if dma_only or stage < 1:
                nc.vector.memset(sum_s[:], 1000.0)
            if dma_only or stage < 4:
                nc.vector.memset(Pst[:], 1000.0)
                nc.vector.memset(Gst[:], 1000.0)
            if dma_only or stage < 3:
                nc.vector.memset(lastp[:], 1000.0)

            for rep in range(repeats):
                # All loads issued up front (write-once xt slots, so the DMA
                # stream has no waits and the transfers pipeline at full BW).
                xts = []
                for nb in range(NB):
                    # Load with f32 -> bf16 cast during DMA (SWDGE).  bf16
                    # keeps the full f32 exponent range, so the sign of every
                    # normal f32 is preserved exactly; only |x| < ~1e-40
                    # could flip, far below this data's 7.5e-8 minimum.
                    xt = xload.tile([P, T], BF16 if cast_dma else F32, tag="xt")
                    nc.gpsimd.dma_start(xt[:], xT[nb * P:(nb + 1) * P, :])
                    xts.append(xt)
                if dma_only:
                    # keep one tiny consumer per tile so nothing is elided
                    for nb in range(NB):
                        nc.vector.tensor_copy(
                            lastp[:, nb:nb + 1], xts[nb][:, 0:1]
                        )
                    continue

                bs = {}

                def emit_sign(nb):
                    if stage < 1:
                        return
                    if nb >= NB - x_dve_split:
                        # DVE variant: b = [x>0] in {0,1} f16 into a fresh
                        # tile; accum -> count directly (host decodes this
                        # column encoding separately).
                        b_t = half.tile([P, T], F16, tag="b")
                        nc.vector.tensor_scalar(
                            b_t[:], xts[nb][:], 0.0, None, op0=A.is_gt,
                            op1=A.add, accum_out=sum_s[:, nb:nb + 1],
                        )
                        bs[nb] = b_t
                        return
                    # In-place s = sign(x); accum -> sum_t sign  (count).
                    nc.scalar.activation(
                        xts[nb][:], xts[nb][:], AF.Sign,
                        accum_out=sum_s[:, nb:nb + 1],
                    )

                # ACT's stream is in-order, so keep the sign passes a couple
                # of groups ahead of the per-group G passes it also runs —
                # blocking on DMA(nb+LOOK) never stalls G(nb) long.
                # phase_mode instead emits ALL sign passes before any
                # accum pass enters ACT's in-order queue: the xt slots are
                # write-once, so the 16 signs are pure runway and ACT never
                # waits on a DVE scan just to start the next sign.
                LOOK = NB if phase_mode else look
                for nb in range(min(LOOK, NB)):
                    emit_sign(nb)

                for nb in range(NB):
                    if nb + LOOK < NB:
                        emit_sign(nb + LOOK)
                    xt = bs.pop(nb) if nb in bs else xts[nb]
                    if stage < 2:
                        continue
                    # v = s * (t+1); negatives are floored away by the scan's
                    # initial=0, so this equals [s>0]*(t+1) post-scan.
                    v = half.tile([P, T], F16, tag="v")
                    if nb < v_pool_split:
                        nc.gpsimd.tensor_tensor(
                            v[:], xt[:], iota_t[:], op=A.mult
                        )
                    elif v_engine == "dve_stt":
                        nc.vector.scalar_tensor_tensor(
                            v[:], xt[:], 0.0, iota_t[:], op0=A.is_gt, op1=A.mult
                        )
                    elif v_engine == "dve_tt":
                        nc.vector.tensor_tensor(
                            v[:], xt[:], iota_t[:], op=A.mult
                        )
                    elif v_engine == "pool_tt":
                        nc.gpsimd.tensor_tensor(
                            v[:], xt[:], iota_t[:], op=A.mult
                        )
                    else:
                        raise ValueError(v_engine)

                    if stage < 3:
                        continue
                    # g = running max of v, floored at 0 (initial=0)
                    g = half.tile([P, T], F16, tag="g")
                    if p_via_scan_accum:
                        # tensor_tensor_scan with a second (accumulator)
                        # output: accum = sum of the scanned outputs = P.
                        # Built manually so Tile tracks the accum write.
                        nc.vector.add_instruction(
                            mybir.InstTensorScalarPtr(
                                name=nc.get_next_instruction_name(),
                                is_tensor_tensor_scan=True,
                                is_scalar_tensor_tensor=True,
                                op0=A.max,
                                op1=A.max,
                                ins=[
                                    nc.vector.lower_ap(v[:]),
                                    nc.vector.lower_ap_or_imm(0.0),
                                    nc.vector.lower_ap(v[:]),
                                ],
                                outs=[
                                    nc.vector.lower_ap(g[:]),
                                    nc.vector.lower_ap(Pst[:, nb:nb + 1]),
                                ],
                            )
                        )
                    else:
                        d1 = (
                            touch[:, 0:1].broadcast_to([P, T])
                            if scan_d1_small else v[:]
                        )
                        nc.vector.tensor_tensor_scan(
                            g[:], v[:], d1, 0.0, op0=A.max,
                            op1=A.bypass if scan_bypass else A.max,
                        )
                    if not p_via_scan_accum and stage >= 4:
                        # P = sum_t g (pass-through overwrites the dead v)
                        if nb < p_split:
                            # ACT Copy is table-free, so no Sign-table thrash
                            nc.scalar.activation(
                                v[:], g[:], AF.Copy,
                                accum_out=Pst[:, nb:nb + 1],
                            )
                        else:
                            nc.vector.tensor_scalar(
                                v[:], g[:], 0.0, None, op0=A.add, op1=A.add,
                                accum_out=Pst[:, nb:nb + 1],
                            )

                    # last+1 = g(T-1)  (before g is clobbered below)
                    nc.vector.tensor_copy(lastp[:, nb:nb + 1], g[:, T - 1:T])
                    if stage < 4:
                        continue

                    # G = sum_t [g>0]; in-place over g (its last use).
                    # g_split: first g_split groups go to DVE even when
                    # g_on_act (load balancing between the two engines).
                    # g_cols_dve: instead split every group's G-pass by
                    # columns — ACT sums [0, T-c), DVE sums [T-c, T); the
                    # final algebra adds the two partial counts.
                    if g_cols_dve:
                        c0 = T - g_cols_dve
                        nc.scalar.activation(
                            g[:, 0:c0], g[:, 0:c0], AF.Sign,
                            accum_out=Gst[:, nb:nb + 1],
                        )
                        nc.vector.tensor_scalar(
                            g[:, c0:T], g[:, c0:T], 1.0, None,
                            op0=A.min, op1=A.add,
                            accum_out=Gst2[:, nb:nb + 1],
                        )
                    elif g_on_act and nb >= g_split:
                        nc.scalar.activation(
                            g[:], g[:], AF.Sign, accum_out=Gst[:, nb:nb + 1]
                        )
                    else:
                        nc.vector.tensor_scalar(
                            g[:], g[:], 1.0, None, op0=A.min, op1=A.add,
                            accum_out=Gst[:, nb:nb + 1],
                        )

                    if final_split and rep == repeats - 1 and nb == NB // 2 - 1:
                        emit_final(0, NB // 2)
                        emitted_halves.add(0)

            if host_final:
                nc.sync.dma_start(sst_o[:], sum_s[:])
                nc.sync.dma_start(gst_o[:], Gst[:])
                nc.sync.dma_start(pst_o[:], Pst[:])
                nc.sync.dma_start(lst_o[:], lastp[:])
            else:
                nc.sync.dma_start(tgt_t[:], tgt[:])

                if final_split:
                    for lo in (0, NB // 2):
                        if lo not in emitted_halves:
                            emit_final(lo, lo + NB // 2)
                else:
                    emit_final(0, NB)

                red = fin.tile([P, 1], F32, tag="red")
                nc.vector.tensor_reduce(red[:], ft("sq")[:], axis=AX.X, op=A.add)
                nc.sync.dma_start(out[:], red[:])

    return nc


def _legalize_waits(nc):
    """Hoist excess sync-waits onto standalone EventSemaphore instructions.

    Hardware instruction encodings hold a single sync-wait (EventSemaphore
    holds two); the deployed tile scheduler sometimes attaches more, which
    walrus codegen rejects ("Too many sync wait commands").  Splitting the
    extra waits into preceding same-engine EventSemaphore ops is exactly
    equivalent: the engine stalls on the standalone waits first.
    """
    f = nc.m.functions[0]
    for blk in f.blocks:
        newlist = []
        for inst in blk.instructions:
            si = inst.sync_info
            tname = type(inst).__name__
            waits = list(si.on_wait) if si is not None else []
            cap = 2 if tname == "InstEventSemaphore" else 1
            if len(waits) <= cap:
                newlist.append(inst)
                continue
            for j, w in enumerate(waits[:-1]):
                es = mybir.InstEventSemaphore(name=f"{inst.name}-hw{j}")
                es.engine = inst.engine
                es.sync_info = mybir.SyncInfo(on_wait=[w], on_update=[])
                newlist.append(es)
            inst.sync_info = mybir.SyncInfo(
                on_wait=[waits[-1]], on_update=list(si.on_update)
            )
            newlist.append(inst)
        blk.instructions = newlist


def _get_nc(**flags):
    key = tuple(sorted(flags.items()))
    if key not in _CACHE:
        nc = _build(**flags)
        _legalize_waits(nc)  # HW path only; CoreSim needs the raw program
        _CACHE[key] = nc
    return _CACHE[key]


FLAGS = dict(v_engine="dve_tt", g_on_act=True, p_split=12,
             scan_bypass=True, half_bufs=6, look=3)


def _host_finish(sst, gst, pst, lst, tgt_pn, x_dve_split=0):
    """Decode per-(partition, group) stats and compute the CV loss terms.

    sst/gst/pst/lst: [P, NB] f32 per-core stats (sst = sum of sign = 2c-T
    for ACT-sign groups, = c directly for the last x_dve_split groups;
    gst = #[g>0], pst = sum g, lst = last+1).  Returns the sum of squared
    errors over this core's P*NB sequences, in float64.
    """
    cnt = (sst.astype(np.float64) + T) * 0.5
    if x_dve_split:
        cnt[:, NB - x_dve_split:] = sst[:, NB - x_dve_split:]
    G = gst.astype(np.float64)
    Pv = pst.astype(np.float64)
    last = lst.astype(np.float64) - 1.0
    first = T - G

    k = cnt - 1.0
    s1 = last - first
    s2 = (2.0 * T * last - last * last - first * first - 2.0 * first
          - 2.0 * (Pv - T))
    mean = s1 / np.maximum(k, 1.0)
    var = (s2 - k * mean * mean) / np.maximum(k - 1.0, 1.0)
    std = np.sqrt(np.maximum(var, 0.0))
    cv = std / np.where(mean > 0.0, mean, 1.0)
    cvs = np.where((cnt >= 3.0) & (mean > 0.0), cv, 10.0)
    d = cvs - tgt_pn
    return float(np.sum(d * d))


def kernel(output_spikes, target_cv):
    x = np.asarray(output_spikes, dtype=np.float32)
    tgt = np.asarray(target_cv, dtype=np.float32)
    assert x.shape == (B, T, N), x.shape

    iota_np = np.broadcast_to(
        (np.arange(T, dtype=np.float32) + 1.0).astype(np.float16), (P, T)
    ).copy()
    tgt_np = np.ascontiguousarray(tgt.reshape(NB, P).T)  # [P, NB]

    host_final = FLAGS.get("host_final", False)
    in_maps = []
    for b in range(B):
        m = {
            "xT": np.ascontiguousarray(x[b].T),  # (N, T)
            "iota": iota_np,
        }
        if not host_final:
            m["tgt"] = tgt_np
        in_maps.append(m)

    nc = _get_nc(**FLAGS)
    res = run_bass_kernel_spmd(nc, in_maps, list(range(B)))

    total = np.float64(0.0)
    if host_final:
        tgt_pn = tgt_np.astype(np.float64)
        for b in range(B):
            r = res.results[b]
            total += _host_finish(
                r["sst"], r["gst"], r["pst"], r["lst"], tgt_pn,
                FLAGS.get("x_dve_split", 0),
            )
    else:
        for b in range(B):
            total += np.asarray(res.results[b]["out"], dtype=np.float64).sum()
    loss = total / float(B * N)
    return np.float32(loss)



# revision 30
# speedup vs baseline: 1.5034x; 1.0971x over previous
"""Trainium2 Bass kernel for the spike-train CV (coefficient of variation) loss.

Problem: for each (batch, neuron) sequence of T=2000 time steps, spikes are
positions where x > 0.  The loss is MSE between per-sequence CV of the
inter-spike intervals (ISIs, unbiased std / mean, penalty 10.0 when fewer
than 3 spikes) and a per-neuron target.

Algorithm (per sequence, all exact integer arithmetic except one fp32 sum):
  s(t)   = sign(x)                                (ACT engine, fp16 out)
  v(t)   = s(t) * (t+1)                           (DVE tensor_tensor, fp16 2x)
  g(t)   = max(0, running max of v)               (DVE tensor_tensor_scan)
         = 1 + (time of last spike <= t), 0 if none
  count  = (sum_t s + T) / 2
  first  = T - sum_t [g>0],   last = g(T-1) - 1
  P      = sum_t g  =>  sum_t prev_incl(t) = P - T
From these, the ISI sum s1 = last-first (telescoping) and the ISI square sum
via the run-length identity:
  R      = sum_{t=first..last} (t - prev_incl(t))
  sum g^2 over internal zero-runs = 2R - Z,  Z = s1+1-count
  s2     = (2R - Z) + 2*s1 - count + 1
then cv = std/mean with torch-style unbiased variance, penalty when count<3.

Sharding: batch dim (B=8) across the 8 cores, embarrassingly parallel; host
transposes each core's slab to (N, T) so time lies along the SBUF free axis
(the scan direction) and sums the 8 per-core partial squared-error sums.
"""

import numpy as np

import concourse.bass as bass
import concourse.tile as tile
from concourse import mybir
from concourse.bass_utils import run_bass_kernel_spmd

B, T, N = 8, 2000, 2048
P = 128                 # SBUF partitions
NB = N // P             # 16 neuron groups per core
F32 = mybir.dt.float32
F16 = mybir.dt.float16
BF16 = mybir.dt.bfloat16
A = mybir.AluOpType
AF = mybir.ActivationFunctionType
AX = mybir.AxisListType

_CACHE = {}


def _build(g_on_act=False, p_via_scan_accum=False, v_engine="dve_stt",
           g_split=0, final_split=False, cast_dma=True, dma_only=False,
           p_split=0, scan_bypass=False, scan_d1_small=False, half_bufs=4, look=2,
           stage=4, repeats=1, host_final=False, v_pool_split=0,
           phase_mode=False, x_dve_split=0, g_cols_dve=0, fresh_dead=0):
    nc = bass.Bass("TRN2", target_bir_lowering=False, debug=False, num_devices=B)

    xT = nc.dram_tensor("xT", [N, T], F32, kind="ExternalInput").ap()
    iota = nc.dram_tensor("iota", [P, T], F16, kind="ExternalInput").ap()
    if host_final:
        sst_o = nc.dram_tensor("sst", [P, NB], F32, kind="ExternalOutput").ap()
        gst_o = nc.dram_tensor("gst", [P, NB], F32, kind="ExternalOutput").ap()
        pst_o = nc.dram_tensor("pst", [P, NB], F32, kind="ExternalOutput").ap()
        lst_o = nc.dram_tensor("lst", [P, NB], F32, kind="ExternalOutput").ap()
    else:
        tgt = nc.dram_tensor("tgt", [P, NB], F32, kind="ExternalInput").ap()
        out = nc.dram_tensor("out", [P, 1], F32, kind="ExternalOutput").ap()

    with tile.TileContext(nc) as tc:
        with (
            tc.tile_pool(name="const", bufs=1) as const_pool,
            tc.tile_pool(name="stats", bufs=1) as stats_pool,
            tc.tile_pool(name="xload", bufs=NB) as xload,
            tc.tile_pool(name="half", bufs=half_bufs) as half,
            tc.tile_pool(name="dead2", bufs=max(fresh_dead, 1)) as dead2,
            tc.tile_pool(name="fin", bufs=1) as fin,
        ):
            iota_t = const_pool.tile([P, T], F16, tag="iota")
            nc.gpsimd.dma_start(iota_t[:], iota[:])
            # Pre-touch the constant tile on DVE so downstream tensor_tensor
            # ops don't need a second (DMA) sync-wait slot — the TT ISA
            # struct only has one.
            touch = const_pool.tile([P, 1], F16, tag="touch")
            nc.vector.tensor_copy(touch[:], iota_t[:, 0:1])
            # tgt is only needed by the final math; load it late so its DMA
            # doesn't stall the first TT of the main loop.  Its consumer
            # (the diff TT) carries the DMA wait itself.
            if not host_final:
                tgt_t = const_pool.tile([P, NB], F32, tag="tgt")

            sum_s = stats_pool.tile([P, NB], F32, tag="sum_s")
            Pst = stats_pool.tile([P, NB], F32, tag="Pst")
            Gst = stats_pool.tile([P, NB], F32, tag="Gst")
            lastp = stats_pool.tile([P, NB], F32, tag="lastp")
            if g_cols_dve:
                Gst2 = stats_pool.tile([P, NB], F32, name="Gst2", tag="Gst2")
            else:
                Gst2 = None

            # ---- final per-neuron algebra on [P, NB] f32 tiles ----
            # Everything reduces algebraically to
            #   s2 = 2T*last - last^2 - first^2 - 2*first - 2*(P - T)
            # (the run-length identity chain collapses), then the torch-style
            # unbiased CV with penalty-10 select.
            tiles = {}

            def ft(tag):
                if tag not in tiles:
                    tiles[tag] = fin.tile([P, NB], F32, name=tag, tag=tag)
                return tiles[tag]

            def emit_final(lo, hi):
                sl = slice(lo, hi)

                def ts(out_t, in_t, s1_, s2_, op0, op1=None):
                    if op1 is None:
                        nc.vector.tensor_scalar(
                            out_t[:, sl], in_t[:, sl], s1_, None, op0=op0
                        )
                    else:
                        nc.vector.tensor_scalar(
                            out_t[:, sl], in_t[:, sl], s1_, s2_, op0=op0, op1=op1
                        )
                    return out_t

                def tt(out_t, a, b, op):
                    nc.vector.tensor_tensor(
                        out_t[:, sl], a[:, sl], b[:, sl], op=op
                    )
                    return out_t

                def stt(out_t, a, scal, b, op0, op1):
                    nc.vector.scalar_tensor_tensor(
                        out_t[:, sl], a[:, sl], scal, b[:, sl], op0=op0, op1=op1
                    )
                    return out_t

                cnt = ts(ft("cnt"), sum_s, float(T), 0.5, A.add, A.mult)
                if g_cols_dve:
                    gsum = tt(ft("gsum"), Gst, Gst2, A.add)
                    first = ts(ft("first"), gsum, -1.0, float(T),
                               A.mult, A.add)
                else:
                    first = ts(ft("first"), Gst, -1.0, float(T),
                               A.mult, A.add)
                last = ts(ft("last"), lastp, -1.0, None, A.add)
                s1 = tt(ft("s1"), last, first, A.subtract)
                k = ts(ft("k"), cnt, -1.0, None, A.add)
                h = ts(ft("h"), Pst, -2.0, 2.0 * T, A.mult, A.add)  # -2*P_full
                e1 = ts(ft("e1"), last, 2.0 * T, None, A.mult)
                bb = tt(ft("bb"), last, last, A.mult)
                aa = tt(ft("aa"), first, first, A.mult)
                g1 = tt(ft("g1"), e1, bb, A.subtract)
                g2 = tt(ft("g2"), g1, aa, A.subtract)
                g3 = ts(ft("g3"), first, -2.0, None, A.mult)
                g4 = tt(ft("g4"), g2, g3, A.add)
                s2t = tt(ft("s2t"), g4, h, A.add)

                maxk = ts(ft("maxk"), k, 1.0, None, A.max)
                invmaxk = ft("invmaxk")
                nc.vector.reciprocal(invmaxk[:, sl], maxk[:, sl])
                mean = tt(ft("mean"), s1, invmaxk, A.mult)
                km1 = ts(ft("km1"), k, -1.0, 1.0, A.add, A.max)
                invkm1 = ft("invkm1")
                nc.vector.reciprocal(invkm1[:, sl], km1[:, sl])

                # k*mean^2 == s1*mean for k>=1 (maxk==k); for the masked
                # k<=0 lanes both stay finite, which is all that matters.
                km2 = tt(ft("km2"), s1, mean, A.mult)
                d = tt(ft("d"), s2t, km2, A.subtract)
                var = tt(ft("var"), d, invkm1, A.mult)
                varc = ts(ft("varc"), var, 0.0, None, A.max)
                std = ft("std")
                nc.scalar.activation(std[:, sl], varc[:, sl], AF.Sqrt)

                dm = ts(ft("dm"), mean, -1.0, None, A.add)
                t4 = stt(ft("t4"), mean, 0.0, dm, A.is_gt, A.mult)
                denom = ts(ft("denom"), t4, 1.0, None, A.add)
                invden = ft("invden")
                nc.vector.reciprocal(invden[:, sl], denom[:, sl])
                cv = tt(ft("cv"), std, invden, A.mult)

                cm = ts(ft("cm"), cv, -10.0, None, A.add)
                t5 = stt(ft("t5"), cnt, 3.0, cm, A.is_ge, A.mult)
                cvs = ts(ft("cvs"), t5, 10.0, None, A.add)

                diff = tt(ft("diff"), cvs, tgt_t, A.subtract)
                tt(ft("sq"), diff, diff, A.mult)

            emitted_halves = set()

            if dma_only or stage < 1:
                nc.vector.memset(sum_s[:], 1000.0)
            if dma_only or stage < 4:
                nc.vector.memset(Pst[:], 1000.0)
                nc.vector.memset(Gst[:], 1000.0)
            if dma_only or stage < 3:
                nc.vector.memset(lastp[:], 1000.0)

            for rep in range(repeats):
                # All loads issued up front (write-once xt slots, so the DMA
                # stream has no waits and the transfers pipeline at full BW).
                xts = []
                for nb in range(NB):
                    # Load with f32 -> bf16 cast during DMA (SWDGE).  bf16
                    # keeps the full f32 exponent range, so the sign of every
                    # normal f32 is preserved exactly; only |x| < ~1e-40
                    # could flip, far below this data's 7.5e-8 minimum.
                    xt = xload.tile([P, T], BF16 if cast_dma else F32, tag="xt")
                    nc.gpsimd.dma_start(xt[:], xT[nb * P:(nb + 1) * P, :])
                    xts.append(xt)
                if dma_only:
                    # keep one tiny consumer per tile so nothing is elided
                    for nb in range(NB):
                        nc.vector.tensor_copy(
                            lastp[:, nb:nb + 1], xts[nb][:, 0:1]
                        )
                    continue

                bs = {}

                def emit_sign(nb):
                    if stage < 1:
                        return
                    if nb >= NB - x_dve_split:
                        # DVE variant: b = [x>0] in {0,1} f16 into a fresh
                        # tile; accum -> count directly (host decodes this
                        # column encoding separately).
                        b_t = half.tile([P, T], F16, tag="b")
                        nc.vector.tensor_scalar(
                            b_t[:], xts[nb][:], 0.0, None, op0=A.is_gt,
                            op1=A.add, accum_out=sum_s[:, nb:nb + 1],
                        )
                        bs[nb] = b_t
                        return
                    # In-place s = sign(x); accum -> sum_t sign  (count).
                    nc.scalar.activation(
                        xts[nb][:], xts[nb][:], AF.Sign,
                        accum_out=sum_s[:, nb:nb + 1],
                    )

                # ACT's stream is in-order, so keep the sign passes a couple
                # of groups ahead of the per-group G passes it also runs —
                # blocking on DMA(nb+LOOK) never stalls G(nb) long.
                # phase_mode instead emits ALL sign passes before any
                # accum pass enters ACT's in-order queue: the xt slots are
                # write-once, so the 16 signs are pure runway and ACT never
                # waits on a DVE scan just to start the next sign.
                LOOK = NB if phase_mode else look
                for nb in range(min(LOOK, NB)):
                    emit_sign(nb)

                for nb in range(NB):
                    if nb + LOOK < NB:
                        emit_sign(nb + LOOK)
                    xt = bs.pop(nb) if nb in bs else xts[nb]
                    if stage < 2:
                        continue
                    # v = s * (t+1); negatives are floored away by the scan's
                    # initial=0, so this equals [s>0]*(t+1) post-scan.
                    v = half.tile([P, T], F16, tag="v")
                    if nb < v_pool_split:
                        nc.gpsimd.tensor_tensor(
                            v[:], xt[:], iota_t[:], op=A.mult
                        )
                    elif v_engine == "dve_stt":
                        nc.vector.scalar_tensor_tensor(
                            v[:], xt[:], 0.0, iota_t[:], op0=A.is_gt, op1=A.mult
                        )
                    elif v_engine == "dve_tt":
                        nc.vector.tensor_tensor(
                            v[:], xt[:], iota_t[:], op=A.mult
                        )
                    elif v_engine == "pool_tt":
                        nc.gpsimd.tensor_tensor(
                            v[:], xt[:], iota_t[:], op=A.mult
                        )
                    else:
                        raise ValueError(v_engine)

                    if stage < 3:
                        continue
                    # g = running max of v, floored at 0 (initial=0)
                    g = half.tile([P, T], F16, tag="g")
                    if p_via_scan_accum:
                        # tensor_tensor_scan with a second (accumulator)
                        # output: accum = sum of the scanned outputs = P.
                        # Built manually so Tile tracks the accum write.
                        nc.vector.add_instruction(
                            mybir.InstTensorScalarPtr(
                                name=nc.get_next_instruction_name(),
                                is_tensor_tensor_scan=True,
                                is_scalar_tensor_tensor=True,
                                op0=A.max,
                                op1=A.max,
                                ins=[
                                    nc.vector.lower_ap(v[:]),
                                    nc.vector.lower_ap_or_imm(0.0),
                                    nc.vector.lower_ap(v[:]),
                                ],
                                outs=[
                                    nc.vector.lower_ap(g[:]),
                                    nc.vector.lower_ap(Pst[:, nb:nb + 1]),
                                ],
                            )
                        )
                    else:
                        d1 = (
                            touch[:, 0:1].broadcast_to([P, T])
                            if scan_d1_small else v[:]
                        )
                        nc.vector.tensor_tensor_scan(
                            g[:], v[:], d1, 0.0, op0=A.max,
                            op1=A.bypass if scan_bypass else A.max,
                        )
                    if not p_via_scan_accum and stage >= 4:
                        # P = sum_t g (pass-through overwrites the dead v)
                        if nb < p_split:
                            # ACT Copy is table-free, so no Sign-table thrash
                            nc.scalar.activation(
                                v[:], g[:], AF.Copy,
                                accum_out=Pst[:, nb:nb + 1],
                            )
                        else:
                            # fresh_dead: write to an isolated dead tile so
                            # this pass doesn't extend v's pool lifetime.
                            pd = (dead2.tile([P, T], F16, name="pd", tag="pd")
                                  if fresh_dead else v)
                            nc.vector.tensor_scalar(
                                pd[:], g[:], 0.0, None, op0=A.add, op1=A.add,
                                accum_out=Pst[:, nb:nb + 1],
                            )

                    # last+1 = g(T-1)  (before g is clobbered below)
                    nc.vector.tensor_copy(lastp[:, nb:nb + 1], g[:, T - 1:T])
                    if stage < 4:
                        continue

                    # G = sum_t [g>0]; in-place over g (its last use).
                    # g_split: first g_split groups go to DVE even when
                    # g_on_act (load balancing between the two engines).
                    # g_cols_dve: instead split every group's G-pass by
                    # columns — ACT sums [0, T-c), DVE sums [T-c, T); the
                    # final algebra adds the two partial counts.
                    if g_cols_dve:
                        c0 = T - g_cols_dve
                        nc.scalar.activation(
                            g[:, 0:c0], g[:, 0:c0], AF.Sign,
                            accum_out=Gst[:, nb:nb + 1],
                        )
                        nc.vector.tensor_scalar(
                            g[:, c0:T], g[:, c0:T], 1.0, None,
                            op0=A.min, op1=A.add,
                            accum_out=Gst2[:, nb:nb + 1],
                        )
                    elif g_on_act and nb >= g_split:
                        nc.scalar.activation(
                            g[:], g[:], AF.Sign, accum_out=Gst[:, nb:nb + 1]
                        )
                    else:
                        nc.vector.tensor_scalar(
                            g[:], g[:], 1.0, None, op0=A.min, op1=A.add,
                            accum_out=Gst[:, nb:nb + 1],
                        )

                    if final_split and rep == repeats - 1 and nb == NB // 2 - 1:
                        emit_final(0, NB // 2)
                        emitted_halves.add(0)

            if host_final:
                nc.sync.dma_start(sst_o[:], sum_s[:])
                nc.sync.dma_start(gst_o[:], Gst[:])
                nc.sync.dma_start(pst_o[:], Pst[:])
                nc.sync.dma_start(lst_o[:], lastp[:])
            else:
                nc.sync.dma_start(tgt_t[:], tgt[:])

                if final_split:
                    for lo in (0, NB // 2):
                        if lo not in emitted_halves:
                            emit_final(lo, lo + NB // 2)
                else:
                    emit_final(0, NB)

                red = fin.tile([P, 1], F32, tag="red")
                nc.vector.tensor_reduce(red[:], ft("sq")[:], axis=AX.X, op=A.add)
                nc.sync.dma_start(out[:], red[:])

    return nc


def _legalize_waits(nc):
    """Hoist excess sync-waits onto standalone EventSemaphore instructions.

    Hardware instruction encodings hold a single sync-wait (EventSemaphore
    holds two); the deployed tile scheduler sometimes attaches more, which
    walrus codegen rejects ("Too many sync wait commands").  Splitting the
    extra waits into preceding same-engine EventSemaphore ops is exactly
    equivalent: the engine stalls on the standalone waits first.
    """
    f = nc.m.functions[0]
    for blk in f.blocks:
        newlist = []
        for inst in blk.instructions:
            si = inst.sync_info
            tname = type(inst).__name__
            waits = list(si.on_wait) if si is not None else []
            cap = 2 if tname == "InstEventSemaphore" else 1
            if len(waits) <= cap:
                newlist.append(inst)
                continue
            for j, w in enumerate(waits[:-1]):
                es = mybir.InstEventSemaphore(name=f"{inst.name}-hw{j}")
                es.engine = inst.engine
                es.sync_info = mybir.SyncInfo(on_wait=[w], on_update=[])
                newlist.append(es)
            inst.sync_info = mybir.SyncInfo(
                on_wait=[waits[-1]], on_update=list(si.on_update)
            )
            newlist.append(inst)
        blk.instructions = newlist


def _get_nc(**flags):
    key = tuple(sorted(flags.items()))
    if key not in _CACHE:
        nc = _build(**flags)
        _legalize_waits(nc)  # HW path only; CoreSim needs the raw program
        _CACHE[key] = nc
    return _CACHE[key]


# p_split=15: one P-accumulation on DVE, the rest on ACT Copy.  The
# p_split in {13..16} plateau measures ~8% faster than the old p_split=12:
# DVE tensor_scalar+accum passes sit directly on the TT->scan critical
# chain and cost far more in makespan than their nominal busy time, so
# almost all P accumulation belongs on ACT despite ACT being the busier
# engine.
FLAGS = dict(v_engine="dve_tt", g_on_act=True, p_split=15,
             scan_bypass=True, half_bufs=6, look=3)


def _host_finish(sst, gst, pst, lst, tgt_pn, x_dve_split=0):
    """Decode per-(partition, group) stats and compute the CV loss terms.

    sst/gst/pst/lst: [P, NB] f32 per-core stats (sst = sum of sign = 2c-T
    for ACT-sign groups, = c directly for the last x_dve_split groups;
    gst = #[g>0], pst = sum g, lst = last+1).  Returns the sum of squared
    errors over this core's P*NB sequences, in float64.
    """
    cnt = (sst.astype(np.float64) + T) * 0.5
    if x_dve_split:
        cnt[:, NB - x_dve_split:] = sst[:, NB - x_dve_split:]
    G = gst.astype(np.float64)
    Pv = pst.astype(np.float64)
    last = lst.astype(np.float64) - 1.0
    first = T - G

    k = cnt - 1.0
    s1 = last - first
    s2 = (2.0 * T * last - last * last - first * first - 2.0 * first
          - 2.0 * (Pv - T))
    mean = s1 / np.maximum(k, 1.0)
    var = (s2 - k * mean * mean) / np.maximum(k - 1.0, 1.0)
    std = np.sqrt(np.maximum(var, 0.0))
    cv = std / np.where(mean > 0.0, mean, 1.0)
    cvs = np.where((cnt >= 3.0) & (mean > 0.0), cv, 10.0)
    d = cvs - tgt_pn
    return float(np.sum(d * d))


def kernel(output_spikes, target_cv):
    x = np.asarray(output_spikes, dtype=np.float32)
    tgt = np.asarray(target_cv, dtype=np.float32)
    assert x.shape == (B, T, N), x.shape

    iota_np = np.broadcast_to(
        (np.arange(T, dtype=np.float32) + 1.0).astype(np.float16), (P, T)
    ).copy()
    tgt_np = np.ascontiguousarray(tgt.reshape(NB, P).T)  # [P, NB]

    host_final = FLAGS.get("host_final", False)
    in_maps = []
    for b in range(B):
        m = {
            "xT": np.ascontiguousarray(x[b].T),  # (N, T)
            "iota": iota_np,
        }
        if not host_final:
            m["tgt"] = tgt_np
        in_maps.append(m)

    nc = _get_nc(**FLAGS)
    res = run_bass_kernel_spmd(nc, in_maps, list(range(B)))

    total = np.float64(0.0)
    if host_final:
        tgt_pn = tgt_np.astype(np.float64)
        for b in range(B):
            r = res.results[b]
            total += _host_finish(
                r["sst"], r["gst"], r["pst"], r["lst"], tgt_pn,
                FLAGS.get("x_dve_split", 0),
            )
    else:
        for b in range(B):
            total += np.asarray(res.results[b]["out"], dtype=np.float64).sum()
    loss = total / float(B * N)
    return np.float32(loss)

